# revision 11
# baseline (speedup 1.0000x reference)
"""Trainium2 Bass kernel for nn_BAKTSimpleX (2-block BAKT transformer).

Self-contained: hardcodes shapes (bs=32, s=512, d=256, dff=1024, nb=2, h=8),
shards batch across 8 NeuronCores (4 per core), returns (x, scores).

Key structure (per core, per batch item b, per block i):
 - residual stream x kept [tok, d] (f32); transposed copies made on PE for
   matmul rhs use (contraction wants features on partitions).
 - q == k and scores S = k.k^T is symmetric: ONE QK matmul serves both the
   row-softmax view (rows=q) and the transposed view (rows=k) for P@V.
 - softmax without max-subtraction (scores bounded); masking by region-limited
   matmuls; strictly-lower/upper 0/1 masks only on 128x128 diagonal blocks.
 - P@V uses unnormalized exp(S); normalization (1/rowsum) applied to the
   [head*dk, q] attention result via a PE-built broadcast tile.
 - LN affines folded into adjacent weights host-side.
"""
import sys
import types

import numpy as np
import ml_dtypes

BS, S, D, DFF, NB, H = 32, 512, 256, 1024, 2, 8
DK = D // H  # 32
NCORES = 8
BL = BS // NCORES  # 4 batch per core
NM = S // 128  # 4 token blocks per batch item
BF16 = ml_dtypes.bfloat16

_COMPILED = [None]


def _install_ntff_hook():
    """Shim antenv.axon_hooks so trace=True can NTFF-profile (optional)."""
    try:
        if "antenv.axon_hooks" not in sys.modules:
            m = types.ModuleType("antenv.axon_hooks")
            hook = [None]
            m.set_axon_ntff_profile_hook = lambda h: hook.__setitem__(0, h)
            m.get_axon_ntff_profile_hook = lambda: hook[0]
            sys.modules["antenv.axon_hooks"] = m
            import antenv

            antenv.axon_hooks = m
        import antenv.axon_hooks as ah

        if ah.get_axon_ntff_profile_hook() is None:
            from trn_agent_boot.trn_boot import _ntff_profile_via_ctypes

            ah.set_axon_ntff_profile_hook(
                _ntff_profile_via_ctypes("/opt/axon/libaxon_pjrt.so")
            )
    except Exception:
        pass


def _cosine_pe():
    import math

    pos = np.arange(S, dtype=np.float32)[:, None]
    div = np.exp(
        np.arange(0, D, 2, dtype=np.float32) * (-math.log(10000.0) / D)
    )
    pe = np.zeros((S, D), np.float32)
    pe[:, 0::2] = np.sin(pos * div)
    pe[:, 1::2] = np.cos(pos * div)
    return pe


def _build_bass():
    import concourse.bass as bass
    import concourse.mybir as mybir
    import concourse.tile as tile
    from concourse import bacc
    from concourse.masks import make_identity

    f32 = mybir.dt.float32
    bf16 = mybir.dt.bfloat16
    AF = mybir.ActivationFunctionType

    nc = bacc.Bacc(None, target_bir_lowering=False)

    def din(name, shape, dt=f32):
        return nc.declare_dram_parameter(name, list(shape), dt, isOutput=False)

    # inputs (per-core)
    x0_d = din("x0", [BL, S, D])                  # residual stream, f32
    x0T_d = din("x0T", [D, BL * S], bf16)         # transposed, bf16
    yT_d = din("yT", [D, BL * S], bf16)
    # per-block weights (prepped on host; same on all cores)
    wk_d, bk_d, wv_d, bvr_d, wo_d, bor_d = [], [], [], [], [], []
    w1_d, b1_d, w2_d, b2r_d, g1r_d = [], [], [], [], []
    for i in range(NB):
        wk_d.append(din(f"wk{i}", [D, D], bf16))      # Wk'.T [din, dout]
        bk_d.append(din(f"bk{i}", [128, 2]))          # per-partition, per do-chunk
        wv_d.append(din(f"wv{i}", [D, D], bf16))      # Wv.T [din, dout]
        bvr_d.append(din(f"bvr{i}", [128, D]))        # bv replicated rows
        wo_d.append(din(f"wo{i}", [D, D], bf16))      # Wo.T [dh, dout]
        bor_d.append(din(f"bor{i}", [128, D]))        # bo'' replicated
        w1_d.append(din(f"w1{i}", [D, DFF], bf16))    # W1g.T [din, dff]
        b1_d.append(din(f"b1{i}", [128, 8]))          # b1' per-partition per chunk
        w2_d.append(din(f"w2{i}", [DFF, D], bf16))    # W2.T [dff, dout]
        b2r_d.append(din(f"b2r{i}", [128, D]))        # b2' replicated
        g1r_d.append(din(f"g1r{i}", [128, D]))        # ln1_g replicated
    g0r_d = din("g0r", [128, D])    # block0 ln2_g replicated (block1 residual)
    goutr_d = din("goutr", [128, D])  # final ln2_g replicated
    boutr_d = din("boutr", [128, D])  # final ln2_b replicated
    lt_d = din("lt01", [128, 128], bf16)   # strictly lower ones
    ut_d = din("ut01", [128, 128], bf16)   # strictly upper ones
    i01_d = din("i01", [128, 128], bf16)   # identity ones
    hsel_d = din("hsel", [2, 8, 128], bf16)

    xout_d = nc.declare_dram_parameter("xout", [BL, S, D], f32, isOutput=True)
    sco_d = nc.declare_dram_parameter("scores", [BL, S, S], f32, isOutput=True)

    from contextlib import ExitStack

    with tile.TileContext(nc) as tc, ExitStack() as es:
        cp = es.enter_context(tc.tile_pool(name="consts", bufs=1))
        wp = es.enter_context(tc.tile_pool(name="work", bufs=2))
        bigp = es.enter_context(tc.tile_pool(name="bigwork", bufs=1))
        pp_sc = es.enter_context(tc.tile_pool(name="pp_sc", bufs=1, space="PSUM"))
        pp_big = es.enter_context(tc.tile_pool(name="pp_big", bufs=2, space="PSUM"))
        pp_sml = es.enter_context(tc.tile_pool(name="pp_sml", bufs=1, space="PSUM"))
        pp_tr = es.enter_context(tc.tile_pool(name="pp_tr", bufs=1, space="PSUM"))

        # ---- load constants ----
        def load(pool, dram, shape, dt, rearr=None, **kw):
            t = pool.tile(list(shape), dt, tag=f"ld_{dram.name}")
            ap = dram[:] if rearr is None else dram.rearrange(rearr, **kw)
            nc.sync.dma_start(t[:], ap)
            return t

        wk = [load(cp, wk_d[i], [128, 2, D], bf16, "(c p) o -> p c o", p=128) for i in range(NB)]
        wv = [load(cp, wv_d[i], [128, 2, D], bf16, "(c p) o -> p c o", p=128) for i in range(NB)]
        wo = [load(cp, wo_d[i], [128, 2, D], bf16, "(c p) o -> p c o", p=128) for i in range(NB)]
        w1 = [load(cp, w1_d[i], [128, 2, DFF], bf16, "(c p) o -> p c o", p=128) for i in range(NB)]
        w2 = [load(cp, w2_d[i], [128, 8, D], bf16, "(c p) o -> p c o", p=128) for i in range(NB)]
        bk = [load(cp, bk_d[i], [128, 2], f32) for i in range(NB)]
        b1 = [load(cp, b1_d[i], [128, 8], f32) for i in range(NB)]
        bvr = [load(cp, bvr_d[i], [128, D], f32) for i in range(NB)]
        bor = [load(cp, bor_d[i], [128, D], f32) for i in range(NB)]
        b2r = [load(cp, b2r_d[i], [128, D], f32) for i in range(NB)]
        g1r = [load(cp, g1r_d[i], [128, D], f32) for i in range(NB)]
        g0r = load(cp, g0r_d, [128, D], f32)
        goutr = load(cp, goutr_d, [128, D], f32)
        boutr = load(cp, boutr_d, [128, D], f32)
        lt01 = load(cp, lt_d, [128, 128], bf16)
        ut01 = load(cp, ut_d, [128, 128], bf16)
        i01 = load(cp, i01_d, [128, 128], bf16)
        hsel = load(cp, hsel_d, [8, 2, 128], bf16, "g h p -> h g p")
        x0T = load(cp, x0T_d, [128, 2, BL * S], bf16, "(c p) t -> p c t", p=128)
        yT = load(cp, yT_d, [128, 2, BL * S], bf16, "(c p) t -> p c t", p=128)

        iden = cp.tile([128, 128], f32)
        make_identity(nc, iden[:])
        zz = cp.tile([128, 384], f32)
        nc.any.memset(zz[:], 0.0)
        eps = cp.tile([128, 1], f32)
        nc.any.memset(eps[:], 1e-5)
        zcol = cp.tile([1, 128], bf16)
        nc.any.memset(zcol[:], 0.0)
        zrow = cp.tile([1, S], bf16)
        nc.any.memset(zrow[:], 0.0)

        # x1T built across block 0, consumed by block 1 k-proj
        x1T = cp.tile([128, 2, BL * S], bf16)
        # gx1 = ln2g(blk0) * x1_raw, the block-1 residual carry (per b tiles)
        gx1_all = cp.tile([128, BL, NM, D], f32)

        for b in range(BL):
            x0 = wp.tile([128, NM, D], f32, tag="x0")
            nc.sync.dma_start(x0[:], x0_d[b].rearrange("(m p) o -> p m o", p=128))
            ts = b * S  # token offset of this batch item

            for i in range(NB):
                xT = x0T if i == 0 else x1T
                # ---- k projection: kT [do(2 chunks of 128), tok(512)] ----
                kT = wp.tile([128, 2, S], bf16, tag="kT")
                for oc in range(2):
                    kp = pp_big.tile([128, S], f32, tag="pbig")
                    for kc in range(2):
                        nc.tensor.matmul(
                            kp[:], wk[i][:, kc, oc * 128:(oc + 1) * 128],
                            xT[:, kc, ts:ts + S],
                            start=(kc == 0), stop=(kc == 1))
                    nc.vector.tensor_scalar_add(kT[:, oc, :], kp[:], bk[i][:, oc:oc + 1])

                # ---- v projection: v [tok(4x128), do(256)] ----
                v = wp.tile([128, NM, D], bf16, tag="v")
                for m in range(NM):
                    vp = pp_sml.tile([128, D], f32, tag="psml")
                    for kc in range(2):
                        nc.tensor.matmul(
                            vp[:], yT[:, kc, ts + m * 128:ts + (m + 1) * 128],
                            wv[i][:, kc, :],
                            start=(kc == 0), stop=(kc == 1))
                    nc.vector.tensor_add(v[:, m, :], vp[:], bvr[i][:])

                # ---- QK^T (symmetric) + exp ----
                E = bigp.tile([128, NM, H, S], bf16, tag="E")
                Dlow = bigp.tile([128, NM, H, 128], bf16, tag="Dlow")
                Dup = bigp.tile([128, NM, H, 128], bf16, tag="Dup")
                den = wp.tile([128, NM, H], f32, tag="den")
                for c in range(NM):
                    for g in range(2):
                        scp = pp_sc.tile([128, 4, S], f32, tag="scp")
                        for j in range(4):
                            h = g * 4 + j
                            nc.tensor.matmul(
                                scp[:, j, :],
                                kT[32 * j:32 * j + 32, g, c * 128:(c + 1) * 128],
                                kT[32 * j:32 * j + 32, g, :],
                                start=True, stop=True,
                                tile_position=(32 * j, 0))
                        nc.scalar.activation(
                            E[:, c, g * 4:(g + 1) * 4, :], scp[:], AF.Exp)
                    # diagonal-block masked copies
                    nc.vector.tensor_mul(
                        Dlow[:, c, :, :], E[:, c, :, c * 128:(c + 1) * 128],
                        lt01[:, None, :].to_broadcast((128, H, 128)))
                    nc.vector.tensor_mul(
                        Dup[:, c, :, :], E[:, c, :, c * 128:(c + 1) * 128],
                        ut01[:, None, :].to_broadcast((128, H, 128)))
                    # denominators (A-view): sum over valid k (k < q)
                    if c == 0:
                        nc.vector.reduce_sum(
                            den[:, c, :], Dlow[:, c, :, :], axis=mybir.AxisListType.X)
                    else:
                        dl = wp.tile([128, H], f32, tag="dl")
                        dr = wp.tile([128, H], f32, tag="dr")
                        nc.vector.reduce_sum(
                            dl[:], E[:, c, :, :c * 128], axis=mybir.AxisListType.X)
                        nc.vector.reduce_sum(
                            dr[:], Dlow[:, c, :, :], axis=mybir.AxisListType.X)
                        nc.vector.tensor_add(den[:, c, :], dl[:], dr[:])
                # q=0 has empty denom: avoid 1/0
                nc.any.memset(den[0:1, 0, :], 1.0)

                # ---- denom -> rdT [8, q] (transposed reciprocal) ----
                rdT = wp.tile([8, S], f32, tag="rdT")
                for c in range(NM):
                    dTp = pp_tr.tile([8, 128], f32, tag="tp")
                    nc.tensor.transpose(dTp[:], den[:, c, :], iden[:])
                    nc.vector.reciprocal(rdT[:, c * 128:(c + 1) * 128], dTp[:])
                nc.any.memset(rdT[:, 0:1], 0.0)  # p row 0 is zeroed
                rdTb = wp.tile([8, S], bf16, tag="rdTb")
                nc.vector.tensor_copy(rdTb[:], rdT[:])

                # ---- P@V (unnormalized, B-view) + normalize ----
                attnT = wp.tile([128, 2, S], bf16, tag="attnT")
                for g in range(2):
                    pvp = pp_big.tile([128, S], f32, tag="pbig")
                    # start the bank across all 128 partitions with a zero outer
                    # product (K=1), so the col-packed M=32 matmuls can all
                    # accumulate into one group (HW start zeroes the whole bank).
                    nc.tensor.matmul(pvp[:], zcol[:], zrow[:],
                                     start=True, stop=False)
                    for j in range(4):
                        h = g * 4 + j
                        for r in range(NM):  # q region r
                            for c in range(r + 1):  # k chunk c <= r
                                if c == r:
                                    rhs = Dup[:, r, h, :]
                                else:
                                    rhs = E[:, c, h, r * 128:(r + 1) * 128]
                                nc.tensor.matmul(
                                    pvp[32 * j:32 * j + 32, r * 128:(r + 1) * 128],
                                    v[:, c, 32 * h:32 * h + 32], rhs,
                                    start=False, stop=False,
                                    tile_position=(0, 32 * j))
                    nc.tensor.matmul(pvp[:], zcol[:], zrow[:],
                                     start=False, stop=True)
                    # rdB[p, q] = rdT[4g + p//32, q] broadcast tile via PE
                    rdBp = pp_big.tile([128, S], f32, tag="pbig")
                    nc.tensor.matmul(rdBp[:], hsel[:, g, :], rdTb[:],
                                     start=True, stop=True)
                    rdB = wp.tile([128, S], f32, tag="rdB")
                    nc.scalar.copy(rdB[:], rdBp[:])
                    nc.vector.tensor_mul(attnT[:, g, :], pvp[:], rdB[:])

                # ---- attn out-proj + residual + LN1 ----
                x_raw = wp.tile([128, NM, D], f32, tag="x_raw")
                for m in range(NM):
                    aop = pp_sml.tile([128, D], f32, tag="psml")
                    for g in range(2):
                        nc.tensor.matmul(
                            aop[:], attnT[:, g, m * 128:(m + 1) * 128],
                            wo[i][:, g, :], start=(g == 0), stop=(g == 1))
                    z1 = wp.tile([128, D], f32, tag="z1")
                    nc.vector.tensor_add(z1[:], aop[:], bor[i][:])
                    if i == 0:
                        nc.vector.tensor_add(z1[:], z1[:], x0[:, m, :])
                    else:
                        nc.vector.tensor_add(z1[:], z1[:], gx1_all[:, b, m, :])
                    bno = wp.tile([128, 6], f32, tag="bno")
                    mv = wp.tile([128, 2], f32, tag="mv")
                    nc.vector.bn_stats(bno[:], z1[:])
                    nc.vector.bn_aggr(mv[:], bno[:])
                    std = wp.tile([128, 1], f32, tag="std")
                    rstd = wp.tile([128, 1], f32, tag="rstd")
                    nc.scalar.activation(std[:], mv[:, 1:2], AF.Sqrt, bias=eps[:])
                    nc.vector.reciprocal(rstd[:], std[:])
                    nc.vector.tensor_scalar(
                        x_raw[:, m, :], z1[:], mv[:, 0:1], rstd[:],
                        mybir.AluOpType.subtract, mybir.AluOpType.mult)

                # ---- transpose x_raw -> xrT (bf16) ----
                xrT = wp.tile([128, 2, S], bf16, tag="xrT")
                for kc in range(2):
                    for m in range(NM):
                        tp = pp_tr.tile([128, 128], f32, tag="tp")
                        nc.tensor.transpose(
                            tp[:], x_raw[:, m, kc * 128:(kc + 1) * 128], iden[:])
                        nc.scalar.copy(xrT[:, kc, m * 128:(m + 1) * 128], tp[:])

                # ---- FFN1 + relu: reluT [dff(8x128), tok] ----
                reluT = bigp.tile([128, 8, S], bf16, tag="reluT")
                for fc in range(8):
                    fp = pp_big.tile([128, S], f32, tag="pbig")
                    for kc in range(2):
                        nc.tensor.matmul(
                            fp[:], w1[i][:, kc, fc * 128:(fc + 1) * 128],
                            xrT[:, kc, :], start=(kc == 0), stop=(kc == 1))
                    nc.scalar.activation(
                        reluT[:, fc, :], fp[:], AF.Relu, bias=b1[i][:, fc:fc + 1])

                # ---- FFN2 + residual + LN2 ----
                for m in range(NM):
                    f2p = pp_sml.tile([128, D], f32, tag="psml")
                    for fc in range(8):
                        nc.tensor.matmul(
                            f2p[:], reluT[:, fc, m * 128:(m + 1) * 128],
                            w2[i][:, fc, :], start=(fc == 0), stop=(fc == 7))
                    z2 = wp.tile([128, D], f32, tag="z2")
                    tg = wp.tile([128, D], f32, tag="tg")
                    nc.vector.tensor_add(z2[:], f2p[:], b2r[i][:])
                    nc.vector.tensor_mul(tg[:], x_raw[:, m, :], g1r[i][:])
                    nc.vector.tensor_add(z2[:], z2[:], tg[:])
                    bno = wp.tile([128, 6], f32, tag="bno")
                    mv = wp.tile([128, 2], f32, tag="mv")
                    nc.vector.bn_stats(bno[:], z2[:])
                    nc.vector.bn_aggr(mv[:], bno[:])
                    std = wp.tile([128, 1], f32, tag="std")
                    rstd = wp.tile([128, 1], f32, tag="rstd")
                    nc.scalar.activation(std[:], mv[:, 1:2], AF.Sqrt, bias=eps[:])
                    nc.vector.reciprocal(rstd[:], std[:])
                    if i == 0:
                        xr2 = wp.tile([128, D], f32, tag="xr2")
                        nc.vector.tensor_scalar(
                            xr2[:], z2[:], mv[:, 0:1], rstd[:],
                            mybir.AluOpType.subtract, mybir.AluOpType.mult)
                        nc.vector.tensor_mul(gx1_all[:, b, m, :], xr2[:], g0r[:])
                        for kc in range(2):
                            tp = pp_tr.tile([128, 128], f32, tag="tp")
                            nc.tensor.transpose(
                                tp[:], xr2[:, kc * 128:(kc + 1) * 128], iden[:])
                            nc.scalar.copy(
                                x1T[:, kc, ts + m * 128:ts + (m + 1) * 128], tp[:])
                    else:
                        xr2 = wp.tile([128, D], f32, tag="xr2")
                        xo = wp.tile([128, D], f32, tag="xo")
                        nc.vector.tensor_scalar(
                            xr2[:], z2[:], mv[:, 0:1], rstd[:],
                            mybir.AluOpType.subtract, mybir.AluOpType.mult)
                        nc.vector.tensor_mul(xo[:], xr2[:], goutr[:])
                        nc.vector.tensor_add(xo[:], xo[:], boutr[:])
                        nc.sync.dma_start(
                            xout_d[b, m * 128:(m + 1) * 128, :], xo[:])

                # ---- scores output (block 1 only) ----
                if i == NB - 1:
                    rdA = wp.tile([128, NM, H], bf16, tag="rdA")
                    Dg = bigp.tile([128, NM, H, 128], bf16, tag="Dg")
                    rdAf = wp.tile([128, NM, H], f32, tag="rdAf")
                    nc.vector.reciprocal(rdAf[:], den[:])
                    nc.vector.tensor_copy(rdA[:], rdAf[:])
                    for c in range(NM):
                        nc.vector.tensor_mul(
                            Dg[:, c, :, :],
                            i01[:, None, :].to_broadcast((128, H, 128)),
                            rdA[:, c, :, None].to_broadcast((128, H, 128)))
                    nc.any.memset(Dg[0:1, 0, :, :], 0.0)
                    for qc in range(NM):
                        w = (qc + 1) * 128
                        sp = pp_big.tile([128, S], f32, tag="pbig")
                        nc.tensor.matmul(sp[:], zcol[:], zrow[:],
                                         start=True, stop=False)
                        for h in range(H):
                            if qc > 0:
                                nc.tensor.matmul(
                                    sp[:, :qc * 128], Dg[:, qc, h, :],
                                    E[:, qc, h, :qc * 128],
                                    start=False, stop=False)
                            nc.tensor.matmul(
                                sp[:, qc * 128:w], Dg[:, qc, h, :],
                                Dlow[:, qc, h, :],
                                start=False, stop=False)
                        nc.tensor.matmul(sp[:], zcol[:], zrow[:],
                                         start=False, stop=True)
                        ssb = wp.tile([128, S], f32, tag="ssb")
                        nc.scalar.mul(ssb[:, :w], sp[:, :w], 1.0 / H)
                        nc.sync.dma_start(
                            sco_d[b, qc * 128:(qc + 1) * 128, :w], ssb[:, :w])
                        if qc < NM - 1:
                            nc.sync.dma_start(
                                sco_d[b, qc * 128:(qc + 1) * 128, w:],
                                zz[:, :S - w])

    nc.compile()
    return nc


def _prep_inputs(q_embed_data, qa_embed_data, Wk, bk, Wv, bv, Wo, bo,
                 ln1_g, ln1_b, W1, b1, W2, b2, ln2_g, ln2_b):
    """Host-side weight folding + per-core input maps."""
    pe = _cosine_pe()
    x0 = q_embed_data + pe[None]   # [BS, S, D] f32
    y0 = qa_embed_data + pe[None]

    csc = 1.0 / np.sqrt(np.sqrt(DK, dtype=np.float64)).astype(np.float32)

    shared = {}
    rep = lambda v: np.repeat(v[None, :].astype(np.float32), 128, axis=0)
    for i in range(NB):
        # fold ln2 of block 0 (g0,b0) into block-1 k-proj; fold 1/dk^.25
        if i == 0:
            Wk_eff = Wk[0] * csc
            bk_eff = bk[0] * csc
        else:
            g0, b0 = ln2_g[0], ln2_b[0]
            Wk_eff = (Wk[1] * g0[None, :]) * csc
            bk_eff = (bk[1] + Wk[1] @ b0) * csc
        shared[f"wk{i}"] = np.ascontiguousarray(Wk_eff.T).astype(BF16)
        shared[f"bk{i}"] = np.ascontiguousarray(
            bk_eff.reshape(2, 128).T).astype(np.float32)
        shared[f"wv{i}"] = np.ascontiguousarray(Wv[i].T).astype(BF16)
        shared[f"bvr{i}"] = rep(bv[i])
        shared[f"wo{i}"] = np.ascontiguousarray(Wo[i].T).astype(BF16)
        bo_eff = bo[i] + (ln2_b[0] if i == 1 else 0.0)
        shared[f"bor{i}"] = rep(bo_eff)
        # fold ln1 affine into W1/b1
        W1g = W1[i] * ln1_g[i][None, :]
        b1_eff = b1[i] + W1[i] @ ln1_b[i]
        shared[f"w1{i}"] = np.ascontiguousarray(W1g.T).astype(BF16)
        shared[f"b1{i}"] = np.ascontiguousarray(
            b1_eff.reshape(8, 128).T).astype(np.float32)
        shared[f"w2{i}"] = np.ascontiguousarray(W2[i].T).astype(BF16)
        shared[f"b2r{i}"] = rep(b2[i] + ln1_b[i])
        shared[f"g1r{i}"] = rep(ln1_g[i])
    shared["g0r"] = rep(ln2_g[0])
    shared["goutr"] = rep(ln2_g[1])
    shared["boutr"] = rep(ln2_b[1])
    lt = np.tril(np.ones((128, 128), np.float32), -1)
    ut = np.triu(np.ones((128, 128), np.float32), 1)
    shared["lt01"] = lt.astype(BF16)
    shared["ut01"] = ut.astype(BF16)
    shared["i01"] = np.eye(128, dtype=np.float32).astype(BF16)
    hsel = np.zeros((2, 8, 128), np.float32)
    for g in range(2):
        for h in range(8):
            for j in range(4):
                if h == 4 * g + j:
                    hsel[g, h, 32 * j:32 * j + 32] = 1.0
    shared["hsel"] = hsel.astype(BF16)

    in_maps = []
    for core in range(NCORES):
        sl = slice(core * BL, (core + 1) * BL)
        xb = x0[sl]    # [BL, S, D]
        yb = y0[sl]
        m = dict(shared)
        m["x0"] = np.ascontiguousarray(xb.astype(np.float32))
        m["x0T"] = np.ascontiguousarray(
            xb.reshape(BL * S, D).T).astype(BF16)
        m["yT"] = np.ascontiguousarray(
            yb.reshape(BL * S, D).T).astype(BF16)
        in_maps.append(m)
    return in_maps


def kernel(**inputs):
    _install_ntff_hook()
    from concourse.bass_utils import run_bass_kernel_spmd

    if _COMPILED[0] is None:
        _COMPILED[0] = _build_bass()
    nc = _COMPILED[0]

    in_maps = _prep_inputs(**{k: np.asarray(v) for k, v in inputs.items()})
    r = run_bass_kernel_spmd(nc, in_maps, list(range(NCORES)))
    x = np.concatenate([r.results[c]["xout"] for c in range(NCORES)], axis=0)
    scores = np.concatenate(
        [r.results[c]["scores"] for c in range(NCORES)], axis=0)
    return x.astype(np.float32), scores.astype(np.float32)
